# revision 20
# baseline (speedup 1.0000x reference)
# GFNet-style block on 8 trn2 NeuronCores, data-parallel over batch.
#
# Per batch element (891 rows x 900 channels):
#   LN1 -> 3D rfftn-filter-irfftn (as two real matmuls against precomputed
#   DFT basis matrices, complex weight applied elementwise on DVE) -> LN2
#   -> PE transpose to feature-major -> low-rank MLP (900->450->1800 gelu
#   ->450->900, biases folded into matmuls/activations) -> residual add.
#
# Precision: bf16 matmul operands, fp32 PSUM accumulation, fp32 LN stats
# and residual.  Host-side numpy folds gamma1 into the complex weight,
# beta1 into a spectral DC ones-row, gamma2/beta2 into u1, v2_b into an
# augmented ones-row of t3.

import os

# the NEFF executes through the axon PJRT plugin; make sure jax can see it
# even if the caller pinned JAX_PLATFORMS to cpu
if os.environ.get("AXON_H4_ENABLED") == "1":
    _jp = os.environ.get("JAX_PLATFORMS")
    if _jp is not None and "axon" not in _jp:
        os.environ["JAX_PLATFORMS"] = "axon," + _jp

import numpy as np
import ml_dtypes

import concourse.bass as bass
import concourse.tile as tile
from concourse import bacc, mybir
from concourse.bass_utils import run_bass_kernel_spmd
from concourse.masks import make_identity

BQ, H, W, D, DR, C = 64, 9, 11, 9, 5, 900
N = H * W * D            # 891
KSP = H * W * DR         # 495 complex spectral bins
KS = 512                 # padded re/im block size
KK = 2 * KS              # 1024 padded spectral rows
NCORES = 8
BL = BQ // NCORES        # 8 batch elements per core
R, HID, RA = 450, 1800, 451
EPS = 1e-5

BF = mybir.dt.bfloat16
F32 = mybir.dt.float32
_BFNP = ml_dtypes.bfloat16


def _chunks(total, size):
    out, o = [], 0
    while o < total:
        out.append((o, min(size, total - o)))
        o += size
    return out


ROW_T = _chunks(N, 128)      # 7 row tiles (last 123)
SPC_T = _chunks(KK, 128)     # 8 spectral tiles
C_T = _chunks(C, 128)        # 8 channel tiles (last 4)
R_T = _chunks(R, 128)        # 4 (last 66)
RA_T = _chunks(RA, 128)      # 4 (last 67)
HID_T = _chunks(HID, 120)    # 15 x 120
NH = [(0, 446), (446, 445)]  # row halves for matmul free dim
CCH = [(0, 450), (450, 450)]  # channel halves


def _host_constants(inputs):
    """Fold params into the matrices the device kernel consumes."""
    cw = np.asarray(inputs["cw"], np.float32)
    g1 = np.asarray(inputs["gamma1"], np.float32)
    b1 = np.asarray(inputs["beta1"], np.float32)
    g2 = np.asarray(inputs["gamma2"], np.float32)
    b2 = np.asarray(inputs["beta2"], np.float32)
    u1 = np.asarray(inputs["u1_w"], np.float32)
    v1 = np.asarray(inputs["v1_w"], np.float32)
    v1b = np.asarray(inputs["v1_b"], np.float32)
    u2 = np.asarray(inputs["u2_w"], np.float32)
    v2 = np.asarray(inputs["v2_w"], np.float32)
    v2b = np.asarray(inputs["v2_b"], np.float32)

    # forward rfftn (ortho) of the 9x11x9 grid as a real matrix [495c, 891]
    eye = np.eye(N, dtype=np.float64).reshape(N, H, W, D)
    F = np.fft.rfftn(eye, axes=(1, 2, 3), norm="ortho").reshape(N, KSP).T
    mfT = np.zeros((N, KK), np.float32)
    mfT[:, 0:KSP] = F.real.T
    mfT[:, KS:KS + KSP] = F.imag.T

    # inverse irfftn (ortho) from (re, im) spectral basis -> [891]
    eyeK = np.eye(KSP, dtype=np.float64).reshape(KSP, H, W, DR)
    Zr = np.fft.irfftn(eyeK, s=(H, W, D), axes=(1, 2, 3), norm="ortho").reshape(KSP, N)
    Zi = np.fft.irfftn(1j * eyeK, s=(H, W, D), axes=(1, 2, 3), norm="ortho").reshape(KSP, N)
    miT = np.zeros((KK, N), np.float32)
    miT[0:KSP] = Zr
    miT[KS:KS + KSP] = Zi

    # beta1's contribution: constant-over-grid filter output, rides spectral
    # row 495 (a padding row) with miT row 495 = ones
    wfull = cw[..., 0] + 1j * cw[..., 1]
    dc = np.fft.rfftn(np.ones((H, W, D, 1)) * b1[None, None, None, :],
                      axes=(0, 1, 2), norm="ortho")
    off1 = np.fft.irfftn(dc * wfull, s=(H, W, D), axes=(0, 1, 2),
                         norm="ortho")[0, 0, 0, :].astype(np.float32)
    miT[KSP, :] = 1.0

    # complex weight with gamma1 folded; [wr;0;wi;0] padded layout so the
    # swapped operand is just a 4-tile rotation
    wcat = np.zeros((KK, C), np.float32)
    wcat[0:KSP] = cw[..., 0].reshape(KSP, C) * g1[None, :]
    wcat[KS:KS + KSP] = cw[..., 1].reshape(KSP, C) * g1[None, :]

    u1pT = (u1 * g2[None, :]).T.copy()        # [900, 450]
    b1p = np.zeros((128, len(R_T)), np.float32)
    bias1 = u1 @ b2
    for j, (o, sz) in enumerate(R_T):
        b1p[:sz, j] = bias1[o:o + sz]
    v1bp = np.zeros((120, len(HID_T)), np.float32)
    for j, (o, sz) in enumerate(HID_T):
        v1bp[:sz, j] = v1b[o:o + sz]
    v2Ta = np.concatenate([v2.T, v2b[None, :]], axis=0)  # [451, 900]

    bf = lambda a: np.ascontiguousarray(a).astype(_BFNP)
    return {
        "mfT": bf(mfT), "miT": bf(miT), "wcat": bf(wcat),
        "off1": bf(off1[None, :]),
        "u1pT": bf(u1pT), "v1T": bf(v1.T), "u2T": bf(u2.T), "v2Ta": bf(v2Ta),
        "b1p": b1p, "v1bp": v1bp,
        "onesrow": np.ones((1, N), _BFNP),
    }


def build_module(bl=BL, gelu_func=None):
    if gelu_func is None:
        gelu_func = mybir.ActivationFunctionType.Gelu
    nc = bacc.Bacc("TRN2", target_bir_lowering=False, debug=False,
                   enable_asserts=False, num_devices=NCORES)

    x_d = nc.dram_tensor("x", [bl, N, C], F32, kind="ExternalInput").ap()
    out_d = nc.dram_tensor("out", [bl, N, C], F32, kind="ExternalOutput").ap()
    mfT_d = nc.dram_tensor("mfT", [N, KK], BF, kind="ExternalInput").ap()
    miT_d = nc.dram_tensor("miT", [KK, N], BF, kind="ExternalInput").ap()
    wcat_d = nc.dram_tensor("wcat", [KK, C], BF, kind="ExternalInput").ap()
    off1_d = nc.dram_tensor("off1", [1, C], BF, kind="ExternalInput").ap()
    u1pT_d = nc.dram_tensor("u1pT", [C, R], BF, kind="ExternalInput").ap()
    v1T_d = nc.dram_tensor("v1T", [R, HID], BF, kind="ExternalInput").ap()
    u2T_d = nc.dram_tensor("u2T", [HID, R], BF, kind="ExternalInput").ap()
    v2Ta_d = nc.dram_tensor("v2Ta", [RA, C], BF, kind="ExternalInput").ap()
    b1p_d = nc.dram_tensor("b1p", [128, len(R_T)], F32, kind="ExternalInput").ap()
    v1bp_d = nc.dram_tensor("v1bp", [120, len(HID_T)], F32, kind="ExternalInput").ap()
    ones_d = nc.dram_tensor("onesrow", [1, N], BF, kind="ExternalInput").ap()

    MULT = mybir.AluOpType.mult

    with tile.TileContext(nc) as tc:
        with (
            tc.tile_pool(name="const", bufs=1) as const,
            tc.tile_pool(name="xin", bufs=2) as xpool,
            tc.tile_pool(name="stat", bufs=8) as stat,
            tc.tile_pool(name="act", bufs=1) as act,
            tc.tile_pool(name="xres", bufs=4) as xres,
            tc.tile_pool(name="psf", bufs=2, space="PSUM") as psf,
            tc.tile_pool(name="psi", bufs=3, space="PSUM") as psi,
            tc.tile_pool(name="psm", bufs=3, space="PSUM") as psm,
        ):
            # ---- persistent constants ----
            def _load(pool, dram, parts, cols, tagp):
                tiles = []
                for i, (o, sz) in enumerate(parts):
                    t = pool.tile([sz, cols], BF, tag=f"{tagp}{i}")
                    nc.sync.dma_start(out=t, in_=dram[o:o + sz, :])
                    tiles.append(t)
                return tiles

            mfT_sb = _load(const, mfT_d, ROW_T, KK, "mfT")
            miT_sb, wcat_sb = [], []

            def load_fwd_consts():
                wcat_sb.extend(_load(const, wcat_d, SPC_T, C, "wc"))
                miT_sb.extend(_load(const, miT_d, SPC_T, N, "miT"))
            u1pT_sb, v1T_sb, u2T_sb, v2Ta_sb = [], [], [], []
            b1p_sb = const.tile([128, len(R_T)], F32, tag="b1p")
            v1bp_sb = const.tile([120, len(HID_T)], F32, tag="v1bp")

            def load_mlp_consts():
                # emitted after batch-0's forward pass so the startup DMA
                # burst doesn't delay the first matmuls
                u1pT_sb.extend(_load(const, u1pT_d, C_T, R, "u1"))
                v1T_sb.extend(_load(const, v1T_d, R_T, HID, "v1"))
                u2T_sb.extend(_load(const, u2T_d, HID_T, R, "u2"))
                v2Ta_sb.extend(_load(const, v2Ta_d, RA_T, C, "v2"))
                nc.sync.dma_start(out=b1p_sb, in_=b1p_d)
                nc.sync.dma_start(out=v1bp_sb, in_=v1bp_d)
            ident = const.tile([128, 128], BF, tag="ident")
            make_identity(nc, ident)
            epst = const.tile([128, 1], F32, tag="eps")
            nc.vector.memset(epst, EPS)

            def ln_scalars(mv, rs, tag):
                """mv [p,2] (mean, var) -> (scale=rsqrt(var+eps), bias=-mean*scale)"""
                sq = stat.tile([128, 1], F32, tag=f"sq{tag}")
                nc.scalar.activation(sq[:rs], mv[:rs, 1:2],
                                     mybir.ActivationFunctionType.Sqrt,
                                     bias=epst[:rs], scale=1.0)
                rcp = stat.tile([128, 1], F32, tag=f"rc{tag}")
                nc.vector.reciprocal(rcp[:rs], sq[:rs])
                nmu = stat.tile([128, 1], F32, tag=f"nm{tag}")
                nc.vector.scalar_tensor_tensor(
                    out=nmu[:rs], in0=mv[:rs, 0:1], scalar=-1.0, in1=rcp[:rs],
                    op0=MULT, op1=MULT)
                return rcp, nmu

            def ln1_fwd_cmult(b):
                # ---- LN1 (row-major, per row tile) ----
                s_tiles = []
                for rt, (ro, rs) in enumerate(ROW_T):
                    xc = xpool.tile([128, C], F32, tag="xc")
                    nc.scalar.dma_start(out=xc[:rs], in_=x_d[b, ro:ro + rs, :])
                    st = stat.tile([128, 2, 6], F32, tag="st1")
                    nc.vector.bn_stats(st[:rs, 0], xc[:rs, 0:450])
                    nc.vector.bn_stats(st[:rs, 1], xc[:rs, 450:900])
                    mv = stat.tile([128, 2], F32, tag="mv1")
                    nc.vector.bn_aggr(mv[:rs], st[:rs])
                    rcp, nmu = ln_scalars(mv, rs, "1")
                    s_t = act.tile([rs, C], BF, tag=f"s{rt}")
                    nc.scalar.activation(s_t, xc[:rs],
                                         mybir.ActivationFunctionType.Identity,
                                         bias=nmu[:rs], scale=rcp[:rs])
                    s_tiles.append(s_t)

                # ---- forward spectral matmul ----
                yf = []
                for m in range(len(SPC_T)):
                    y_t = act.tile([128, C], BF, tag=f"yf{m}")
                    for ch, (co, cs) in enumerate(CCH):
                        ps = psf.tile([128, 512], F32, tag="ft", name="psfwd")
                        for kt, (ro, rs) in enumerate(ROW_T):
                            nc.tensor.matmul(
                                ps[:, 0:cs],
                                mfT_sb[kt][:, m * 128:(m + 1) * 128],
                                s_tiles[kt][:, co:co + cs],
                                start=(kt == 0), stop=(kt == len(ROW_T) - 1))
                        nc.scalar.activation(y_t[:, co:co + cs], ps[:, 0:cs],
                                             mybir.ActivationFunctionType.Copy)
                    yf.append(y_t)

                # ---- complex weight multiply ----
                p2 = []
                for j in range(4):
                    p_t = act.tile([128, C], BF, tag=f"p2{j}")
                    nc.gpsimd.tensor_mul(p_t, yf[j], wcat_sb[j + 4])   # wi*re
                    p2.append(p_t)
                for j in range(4):
                    nc.vector.tensor_mul(yf[j], yf[j], wcat_sb[j])     # wr*re
                for j in range(4):
                    u = act.tile([128, C], BF, tag=f"u{j % 2}")
                    nc.gpsimd.tensor_mul(u, yf[j + 4], wcat_sb[j + 4])       # wi*im
                    nc.vector.tensor_mul(yf[j + 4], yf[j + 4], wcat_sb[j])   # wr*im
                    nc.vector.tensor_sub(yf[j], yf[j], u)              # re2
                    nc.vector.tensor_add(p2[j], p2[j], yf[j + 4])      # im2
                nc.sync.dma_start(out=yf[3][111:112, :], in_=off1_d)
                return [yf[0], yf[1], yf[2], yf[3], p2[0], p2[1], p2[2], p2[3]]

            def inv_ln2_transpose_l1(yf2):
                # ---- inverse spectral matmul + LN2 (row-major out) ----
                z0_tiles = []
                for rt, (ro, rs) in enumerate(ROW_T):
                    half = []
                    for ch, (co, cs) in enumerate(CCH):
                        ps = psi.tile([128, 450], F32, tag="iv", name="psinv")
                        for kt in range(8):
                            nc.tensor.matmul(
                                ps[:rs, 0:cs],
                                miT_sb[kt][:, ro:ro + rs],
                                yf2[kt][:, co:co + cs],
                                start=(kt == 0), stop=(kt == 7))
                        half.append(ps)
                    st = stat.tile([128, 2, 6], F32, tag="st2")
                    nc.vector.bn_stats(st[:rs, 0], half[0][:rs, 0:450])
                    nc.vector.bn_stats(st[:rs, 1], half[1][:rs, 0:450])
                    mv = stat.tile([128, 2], F32, tag="mv2")
                    nc.vector.bn_aggr(mv[:rs], st[:rs])
                    rcp, nmu = ln_scalars(mv, rs, "2")
                    z_t = act.tile([rs, C], BF, tag=f"z{rt}")
                    for ch, (co, cs) in enumerate(CCH):
                        nc.scalar.activation(z_t[:, co:co + cs], half[ch][:rs, 0:cs],
                                             mybir.ActivationFunctionType.Identity,
                                             bias=nmu[:rs], scale=rcp[:rs])
                    z0_tiles.append(z_t)

                # ---- PE transpose z0 -> z0T ----
                z0T = []
                for ct, (co, cs) in enumerate(C_T):
                    zt = act.tile([cs, N], BF, tag=f"zt{ct}")
                    for g, rts in enumerate([(0, 1, 2, 3), (4, 5, 6)]):
                        ps = psf.tile([128, 4, 128], BF, tag="ft")
                        for rj, rt in enumerate(rts):
                            ro, rs = ROW_T[rt]
                            nc.tensor.transpose(ps[:cs, rj, :rs],
                                                z0_tiles[rt][:, co:co + cs],
                                                ident[:rs, :rs])
                        if g == 0:
                            nc.vector.tensor_copy(
                                zt[:, 0:512].rearrange("p (a b) -> p a b", a=4, b=128),
                                ps[:cs, 0:4, :])
                        else:
                            nc.vector.tensor_copy(
                                zt[:, 512:768].rearrange("p (a b) -> p a b", a=2, b=128),
                                ps[:cs, 0:2, :])
                            nc.vector.tensor_copy(zt[:, 768:891], ps[:cs, 2, 0:123])
                    z0T.append(zt)

                # ---- MLP layer 1 ----
                t1 = []
                for m, (mo, ms) in enumerate(R_T):
                    t_t = act.tile([ms, N], BF, tag=f"t1_{m}")
                    for nh, (no, ns) in enumerate(NH):
                        ps = psm.tile([128, 446], F32, tag="mm", name="psl1")
                        for kt, (ko, ks) in enumerate(C_T):
                            nc.tensor.matmul(ps[:ms, 0:ns],
                                             u1pT_sb[kt][:, mo:mo + ms],
                                             z0T[kt][:, no:no + ns],
                                             start=(kt == 0), stop=(kt == len(C_T) - 1))
                        nc.scalar.activation(t_t[:, no:no + ns], ps[:ms, 0:ns],
                                             mybir.ActivationFunctionType.Identity,
                                             bias=b1p_sb[:ms, m:m + 1], scale=1.0)
                    t1.append(t_t)
                return t1

            def xres_dma(b):
                # first 4 x-reload DMAs issued at the head of the iteration
                # so their gpsimd-queue triggers precede the cmult muls and
                # the residual adds never stall the Vector queue
                xrs = []
                for rt, (ro, rs) in enumerate(ROW_T[:4]):
                    xr = xres.tile([128, C], F32, tag="xr", name="xr")
                    nc.gpsimd.dma_start(out=xr[:rs], in_=x_d[b, ro:ro + rs, :])
                    xrs.append(xr)
                return xrs

            def mlp_tail(b, t1, xrs):
                # ---- MLP layers 2+3, one row-half at a time ----
                t3 = []
                for m, (mo, ms) in enumerate(R_T):
                    sz = ms + 1 if m == len(R_T) - 1 else ms
                    t3.append(act.tile([sz, N], BF, tag=f"t3_{m}", name=f"t3_{m}"))
                nc.sync.dma_start(out=t3[-1][RA_T[-1][1] - 1:RA_T[-1][1], :],
                                  in_=ones_d)
                for nh, (no, ns) in enumerate(NH):
                    t2h = []
                    for m, (mo, ms) in enumerate(HID_T):
                        t_t = act.tile([ms, 446], BF, tag=f"t2_{m}")
                        ps = psm.tile([128, 446], F32, tag="mm")
                        for kt, (ko, ks) in enumerate(R_T):
                            nc.tensor.matmul(ps[:ms, 0:ns],
                                             v1T_sb[kt][:, mo:mo + ms],
                                             t1[kt][:, no:no + ns],
                                             start=(kt == 0), stop=(kt == len(R_T) - 1))
                        nc.scalar.activation(t_t[:, 0:ns], ps[:ms, 0:ns],
                                             gelu_func,
                                             bias=v1bp_sb[:ms, m:m + 1], scale=1.0)
                        t2h.append(t_t)
                    for m, (mo, ms) in enumerate(R_T):
                        ps = psm.tile([128, 446], F32, tag="mm")
                        for kt, (ko, ks) in enumerate(HID_T):
                            nc.tensor.matmul(ps[:ms, 0:ns],
                                             u2T_sb[kt][:, mo:mo + ms],
                                             t2h[kt][:, 0:ns],
                                             start=(kt == 0), stop=(kt == len(HID_T) - 1))
                        nc.vector.tensor_copy(t3[m][:ms, no:no + ns], ps[:ms, 0:ns])

                # ---- MLP layer 4 + residual ----
                for rt, (ro, rs) in enumerate(ROW_T):
                    if rt < 4:
                        xr = xrs[rt]
                    else:
                        xr = xres.tile([128, C], F32, tag="xr", name="xr")
                        nc.gpsimd.dma_start(out=xr[:rs], in_=x_d[b, ro:ro + rs, :])
                    for ch, (co, cs) in enumerate(CCH):
                        ps = psm.tile([128, 450], F32, tag="mm", name="psl4")
                        for kt, (ko, ks) in enumerate(RA_T):
                            nc.tensor.matmul(ps[:rs, 0:cs],
                                             t3[kt][:, ro:ro + rs],
                                             v2Ta_sb[kt][:, co:co + cs],
                                             start=(kt == 0), stop=(kt == len(RA_T) - 1))
                        nc.vector.tensor_add(xr[:rs, co:co + cs], xr[:rs, co:co + cs],
                                             ps[:rs, 0:cs])
                    nc.sync.dma_start(out=out_d[b, ro:ro + rs, :], in_=xr[:rs])

            # software pipeline: batch b-1's MLP tail is emitted between
            # FWD(b) and INV(b) so PE has work during the DVE/GpSimd
            # complex-multiply and LN chains
            pending = None
            for b in range(bl):
                if b == 0:
                    load_fwd_consts()
                xrs = xres_dma(pending[0]) if pending is not None else None
                yf2 = ln1_fwd_cmult(b)
                if b == 0:
                    load_mlp_consts()
                if pending is not None:
                    mlp_tail(pending[0], pending[1], xrs)
                t1 = inv_ln2_transpose_l1(yf2)
                pending = (b, t1)
            mlp_tail(pending[0], pending[1], xres_dma(pending[0]))

    nc.compile()
    return nc


_CACHE = {}


def kernel(**inputs):
    if "nc" not in _CACHE:
        _CACHE["nc"] = build_module(BL)
    nc = _CACHE["nc"]
    consts = _host_constants(inputs)
    x = np.ascontiguousarray(np.asarray(inputs["x"], np.float32))
    in_maps = []
    for c in range(NCORES):
        m = {"x": np.ascontiguousarray(x[c * BL:(c + 1) * BL])}
        m.update(consts)
        in_maps.append(m)
    res = run_bass_kernel_spmd(nc, in_maps, core_ids=list(range(NCORES)))
    out = np.concatenate([r["out"] for r in res.results], axis=0)
    return out.astype(np.float32)

